# revision 64
# baseline (speedup 1.0000x reference)
"""Trainium2 Bass kernel for a dense transformer block (B=8, N=1024, C=768, H=12).

Sharding: data-parallel over batch -- one batch element per NeuronCore (8 cores),
weights replicated, no collectives.

Per-core dataflow (x_b: [1024, 768]):
  LN1 (bn_stats on vector, normalize on gpsimd, bf16 out) -> bf16 transposes
  -> feature-major hT in two token halves (QKV starts after the first half)
  -> QKV bf16: q,k feature-major [64, 1024]/head; V token-major, cast fp8,
     packed two key-tiles per tile for DoubleRow AV (ones column for sums;
     v-bias folded into the proj bias on host)
  -> attention, software-pipelined across head pairs (scores/exp of pair p
     overlap AV of pair p-1; the scalar-engine exp stream is the bottleneck):
     scores^T = kT.T @ qT (bf16, scale folded into Wq), rel-bias Toeplitz band
     added on the PE via identity matmul, exp(s-2) -> fp8 (shift keeps e4m3 in
     range; softmax normalization cancels it), DoubleRow fp8 AV + sums row,
     per-pair 1/sums (DVE approx reciprocal) and aT normalize via sel-matmul
     broadcast
  -> token-major proj (normalized aT chunks stationary, wprojT moving, bias via
     bf16 rank-1 matmul) fused + staggered with residual add, LN2, and bf16
     transposes to fp8 h2T halves, so the PE never waits on the vector chain
  -> MLP in fp8 e4m3 with DoubleRow (weights scaled x32 on host, compensated in
     the gelu scale and the fc2 drain): fc1 pairs two 128-contractions per
     matmul, gelu -> fp8 pairs, fc2 pairs two r-tiles
  -> bf16 transposes -> +x residual -> out.
"""

import os

import numpy as np

B, N, C, H, D = 8, 1024, 768, 12, 64
NT = N // 128   # 8 token tiles
KT = C // 128   # 6 feature tiles
F1 = 4 * C      # 3072
RT = F1 // 128  # 24
W = 2 * N - 1   # 2047 toeplitz band width
EPS = 1e-5
FP8S = 32.0  # fp8 weight scale for the MLP

LAST_RESULTS = None  # stash of the last BassKernelResults (for test.py)

_NC_CACHE = {}


def _build_nc(reps=1):
    from contextlib import ExitStack

    import concourse.bacc as bacc
    import concourse.tile as tile
    from concourse import masks, mybir

    f32 = mybir.dt.float32
    f32r = mybir.dt.float32r
    bf16 = mybir.dt.bfloat16
    f8 = mybir.dt.float8e4

    def R(ap):
        return ap.bitcast(f32r)
    AF = mybir.ActivationFunctionType
    AX = mybir.AxisListType
    OP = mybir.AluOpType

    nc = bacc.Bacc(
        "TRN2",
        target_bir_lowering=False,
        debug=False,
        enable_asserts=False,
        num_devices=8,
    )

    x_d = nc.dram_tensor("x", [N, C], f32, kind="ExternalInput").ap()
    wqkv_d = nc.dram_tensor("wqkvT", [C, 3 * C], bf16, kind="ExternalInput").ap()
    bqkv_d = nc.dram_tensor("bqkv", [1, 3 * C], f32, kind="ExternalInput").ap()
    wproj_d = nc.dram_tensor("wprojT", [C, C], bf16, kind="ExternalInput").ap()
    bproj_d = nc.dram_tensor("bproj", [1, C], f32, kind="ExternalInput").ap()
    wfc1_d = nc.dram_tensor("wfc1t", [RT, 128, C], f8, kind="ExternalInput").ap()
    bfc1_d = nc.dram_tensor("bfc1", [1, F1], f32, kind="ExternalInput").ap()
    wfc2_d = nc.dram_tensor("wfc2T", [F1, C], f8, kind="ExternalInput").ap()
    bfc2_d = nc.dram_tensor("bfc2", [1, C], f32, kind="ExternalInput").ap()
    rb_d = nc.dram_tensor("rband", [H, 128, W], bf16, kind="ExternalInput").ap()
    sel_d = nc.dram_tensor("sel", [2, 128], f32, kind="ExternalInput").ap()
    out_d = nc.dram_tensor("out", [N, C], f32, kind="ExternalOutput").ap()

    with tile.TileContext(nc) as tc, ExitStack() as ctx:
        # ---------------- kernel-wide pools (opened first, closed last: LIFO ok)
        cpool = ctx.enter_context(tc.tile_pool(name="const", bufs=1))
        identb = cpool.tile([128, 128], bf16, tag="identb")
        masks.make_identity(nc, identb[:])
        sel = cpool.tile([2, 128], f32, tag="sel")
        nc.sync.dma_start(sel[:], sel_d[:])
        epsc = cpool.tile([128, 1], f32, tag="eps")
        nc.any.memset(epsc[:], EPS)
        # exp(s-2) shift keeps fp8 probabilities below e4m3 max; the softmax
        # normalization cancels the factor exactly
        m2c = cpool.tile([128, 1], f32, tag="m2c")
        nc.any.memset(m2c[:], -2.0)
        bqkv_sb = cpool.tile([128, 18], f32, tag="bqkv")
        nc.sync.dma_start(bqkv_sb[:], bqkv_d[0].rearrange("(a p) -> p a", p=128))
        onesP = cpool.tile([1, 128], bf16, tag="onesP")
        nc.any.memset(onesP[:], 1.0)
        # proj bias as a row (token-major proj adds it via rank-1 matmul)
        bprow_f = cpool.tile([1, C], f32, tag="bprowf")
        nc.sync.dma_start(bprow_f[:], bproj_d[:])
        bprow = cpool.tile([1, C], bf16, tag="bprow")
        nc.vector.tensor_copy(bprow[:], bprow_f[:])
        bfc1_sb = cpool.tile([128, RT], f32, tag="bfc1")
        nc.sync.dma_start(bfc1_sb[:], bfc1_d[0].rearrange("(a p) -> p a", p=128))
        bfc2_sb = cpool.tile([128, 6], f32, tag="bfc2")
        nc.sync.dma_start(bfc2_sb[:], bfc2_d[0].rearrange("(a p) -> p a", p=128))

        stat = ctx.enter_context(tc.tile_pool(name="stat", bufs=8))
        # chain pool: big buffers with slot-cycling via shared tags
        chain = ctx.enter_context(tc.tile_pool(name="chain", bufs=1))

        def fm_tile(name, dt=f32):
            return chain.tile([128, N], dt, tag="fm1024", bufs=14, name=name)

        def layernorm(dst_ap, src_ap):
            """dst = (src - mean(src)) * rsqrt(var(src) + eps)."""
            st6 = stat.tile([128, 12], f32, tag="st6", name="st6")
            nc.vector.bn_stats(st6[:, 0:6], src_ap[:, 0 : C // 2])
            nc.vector.bn_stats(st6[:, 6:12], src_ap[:, C // 2 : C])
            mv = stat.tile([128, 2], f32, tag="mv", name="mv")
            nc.vector.bn_aggr(mv[:], st6[:].rearrange("p (g s) -> p g s", s=6))
            sd = stat.tile([128, 1], f32, tag="sd", name="sd")
            nc.scalar.activation(sd[:], mv[:, 1:2], AF.Sqrt, bias=epsc[:])
            rstd = stat.tile([128, 1], f32, tag="rstd", name="rstd")
            nc.vector.reciprocal(rstd[:], sd[:])
            nmr = stat.tile([128, 1], f32, tag="nmr", name="nmr")
            nc.vector.tensor_scalar(
                nmr[:], mv[:, 0:1], rstd[:], -1.0, op0=OP.mult, op1=OP.mult
            )
            nc.gpsimd.tensor_scalar(
                dst_ap, src_ap, rstd[:], nmr[:], op0=OP.mult, op1=OP.add
            )

        for _rep in range(reps):
            # persistent per-batch state
            xs = [chain.tile([128, C], f32, tag="x", bufs=NT, name=f"x{t}") for t in range(NT)]
            # feature-major LN1 output, split into token halves (tile-level deps
            # let QKV qc=0 start after 4 token tiles), viewed as [p, ct, tok]
            hTh = [
                chain.tile([128, KT * 512], bf16, tag=f"hT{h}", bufs=1, name=f"hT{h}")
                for h in range(2)
            ]
            hT3h = [t[:].rearrange("p (c t) -> p c t", t=512) for t in hTh]
            # V in fp8, two key-tiles packed per tile for DoubleRow AV; padded
            # to 128 cols/head so the dual-fp8 ldweights sees the same [128,2,128]
            # shape as the MLP (pad rows of the AV output are never read)
            vpair = [
                chain.tile(
                    [128, 2 * H * 128], f8, tag="vaug", bufs=NT // 2, name=f"vp{t}"
                )
                for t in range(NT // 2)
            ]
            # s2_all[o, hp*N + q] = 1/softmax_sum(head 2hp+o, query q); partitions
            # 0:2 so it can feed sel-matmuls directly as the moving operand
            s2_all = chain.tile([2, KT * N], f32, tag="s2all", bufs=1, name="s2all")

            # ---------------- phase A+B: load x, LN1, transpose -> hT
            with tc.tile_pool(name="psB", bufs=3, space="PSUM") as psB:
                for t in range(NT):
                    nc.sync.dma_start(xs[t][:], x_d[t * 128 : (t + 1) * 128, :])
                    h1 = chain.tile([128, C], bf16, tag="hln", bufs=3, name=f"h1_{t}")
                    layernorm(h1[:], xs[t][:])
                    ps = psB.tile([128, C], bf16, tag="tp", name="psb")
                    for ct in range(KT):
                        nc.tensor.transpose(
                            ps[:, ct * 128 : (ct + 1) * 128],
                            h1[:, ct * 128 : (ct + 1) * 128],
                            identb[:],
                        )
                    nc.scalar.activation(
                        hT3h[t // 4][:, :, (t % 4) * 128 : (t % 4 + 1) * 128],
                        ps[:].rearrange("p (c q) -> p c q", q=128),
                        AF.Copy,
                    )

            # ---------------- phase C: QKV
            qkT = [fm_tile(f"qkT{i}", bf16) for i in range(12)]
            with tc.tile_pool(name="wqkv", bufs=KT) as wq_pool:
                wq = []
                for ct in range(KT):
                    wt = wq_pool.tile([128, 3 * C], bf16, tag="wq", name=f"wq{ct}")
                    nc.sync.dma_start(wt[:], wqkv_d[ct * 128 : (ct + 1) * 128, :])
                    wq.append(wt)
                with tc.tile_pool(name="psC", bufs=3, space="PSUM") as psC:
                    for qc in range(2):
                        # q,k feature-major for this token half
                        for jt in range(12):
                            ps = psC.tile([128, 512], f32, tag="ps", name="psc")
                            for ct in range(KT):
                                nc.tensor.matmul(
                                    ps[:],
                                    wq[ct][:, jt * 128 : (jt + 1) * 128],
                                    hT3h[qc][:, ct, :],
                                    start=(ct == 0),
                                    stop=(ct == KT - 1),
                                )
                            nc.vector.tensor_scalar_add(
                                qkT[jt][:, qc * 512 : (qc + 1) * 512],
                                ps[:],
                                bqkv_sb[:, jt : jt + 1],
                            )
                        # v token-major (bias folded into proj bias on host)
                        for t in range(qc * 4, qc * 4 + 4):
                            vview = vpair[t // 2][:].rearrange(
                                "p (i h e) -> p i h e", i=2, e=128
                            )[:, t % 2]
                            for vc in range(2):
                                ps = psC.tile([128, 384], f32, tag="psv", bufs=2, name="psv")
                                for ct in range(KT):
                                    nc.tensor.matmul(
                                        ps[:],
                                        hT3h[qc][:, ct, (t % 4) * 128 : (t % 4 + 1) * 128],
                                        wq[ct][:, 2 * C + vc * 384 : 2 * C + (vc + 1) * 384],
                                        start=(ct == 0),
                                        stop=(ct == KT - 1),
                                    )
                                nc.vector.tensor_copy(
                                    vview[:, vc * 6 : (vc + 1) * 6, 0:64],
                                    ps[:].rearrange("p (h e) -> p h e", e=64),
                                )
                            nc.any.memset(vview[:, :, 64:65], 1.0)

            # ---------------- phase D: attention
            aT = [fm_tile(f"aT{i}", bf16) for i in range(KT)]
            with (
                tc.tile_pool(name="rbp", bufs=H) as rbp,
                tc.tile_pool(name="ptp", bufs=17) as ptp,
                tc.tile_pool(name="srowp", bufs=2) as srowp,
                tc.tile_pool(name="oddp", bufs=3) as oddp,
                tc.tile_pool(name="psS", bufs=2, space="PSUM") as psS,
                tc.tile_pool(name="psAV", bufs=2, space="PSUM") as psAV,
                tc.tile_pool(name="psNorm", bufs=2, space="PSUM") as psN,
            ):
                # prefetch all rel-bias bands up front
                rb_all = []
                for h in range(H):
                    rb = rbp.tile([128, W], bf16, tag="rb", name=f"rb{h}")
                    nc.gpsimd.dma_start(rb[:], rb_d[h])
                    rb_all.append(rb)
                DRa = mybir.MatmulPerfMode.DoubleRow

                def scores_block(hp):
                    # head pair (2hp, 2hp+1): even head at partitions 0:64, odd
                    # at 64:128 of the same qkT tiles; exp -> fp8 pair tiles
                    rbs = [rb_all[2 * hp], rb_all[2 * hp + 1]]
                    pts = [[None] * 4, [None] * 4]
                    for kc in range(NT):
                        pss = []
                        for odd in range(2):
                            ro = odd * 64
                            ps = psS.tile([128, 1024], f32, tag="ps", name="pss")
                            for qc in range(2):
                                nc.tensor.matmul(
                                    ps[:, qc * 512 : (qc + 1) * 512],
                                    qkT[6 + hp][ro : ro + 64, kc * 128 : (kc + 1) * 128],
                                    qkT[hp][ro : ro + 64, qc * 512 : (qc + 1) * 512],
                                    start=True,
                                    stop=False,
                                )
                            pss.append(ps)
                        for odd in range(2):
                            ps = pss[odd]
                            for qc in range(2):
                                off = 1023 - kc * 128 + qc * 512
                                nc.tensor.matmul(
                                    ps[:, qc * 512 : (qc + 1) * 512],
                                    identb[:],
                                    rbs[odd][:, off : off + 512],
                                    start=False,
                                    stop=True,
                                )
                            if kc % 2 == 0:
                                pts[odd][kc // 2] = ptp.tile(
                                    [128, 2048], f8, tag="pt", name="pt"
                                )
                            ptv = pts[odd][kc // 2][:].rearrange(
                                "p (i q) -> p i q", i=2
                            )
                            nc.scalar.activation(
                                ptv[:, kc % 2, :], ps[:], AF.Exp, bias=m2c[:]
                            )
                    return pts

                def av_block(hp, pts):
                    for odd in range(2):
                        h = 2 * hp + odd
                        for qc in range(2):
                            pav = psAV.tile([128, 512], f32, tag="pav", name="pav")
                            for pr in range(4):
                                ptv = pts[odd][pr][:].rearrange(
                                    "p (i q) -> p i q", i=2
                                )
                                nc.tensor.matmul(
                                    pav[:, :],
                                    vpair[pr][:].rearrange(
                                        "p (i x) -> p i x", i=2
                                    )[:, :, h * 128 : (h + 1) * 128],
                                    ptv[:, :, qc * 512 : (qc + 1) * 512],
                                    start=(pr == 0),
                                    stop=(pr == 3),
                                    perf_mode=DRa,
                                )
                            if odd:
                                tmp = oddp.tile(
                                    [128, 512], bf16, tag="odd", name="avodd"
                                )
                                nc.vector.tensor_copy(tmp[0:64, :], pav[0:64, :])
                                nc.sync.dma_start(
                                    aT[hp][64:128, qc * 512 : (qc + 1) * 512],
                                    tmp[0:64, :],
                                )
                            else:
                                nc.vector.tensor_copy(
                                    aT[hp][0:64, qc * 512 : (qc + 1) * 512],
                                    pav[0:64, :],
                                )
                            srow = srowp.tile([128, 512], f32, tag="srow", name="srow")
                            nc.vector.tensor_copy(srow[64:65, :], pav[64:65, :])
                            nc.sync.dma_start(
                                s2_all[
                                    odd : odd + 1,
                                    hp * N + qc * 512 : hp * N + (qc + 1) * 512,
                                ],
                                srow[64:65, :],
                            )
                    # in-place 1/sums for this head pair (custom DVE op needs
                    # base partition 0, hence on s2_all rather than pav)
                    nc.vector.reciprocal_approx_fast(
                        s2_all[0:2, hp * N : (hp + 1) * N],
                        s2_all[0:2, hp * N : (hp + 1) * N],
                    )
                    # normalize this pair's aT right away (overlaps next pair)
                    for qc in range(2):
                        psb = psN.tile([128, 512], f32, tag="psn", name="psn")
                        nc.tensor.matmul(
                            psb[:],
                            sel[:],
                            s2_all[
                                0:2, hp * N + qc * 512 : hp * N + (qc + 1) * 512
                            ],
                            start=True,
                            stop=True,
                        )
                        nc.vector.tensor_mul(
                            aT[hp][:, qc * 512 : (qc + 1) * 512],
                            aT[hp][:, qc * 512 : (qc + 1) * 512],
                            psb[:],
                        )

                # software-pipeline head pairs: scores/exp of hp overlap the
                # AV matmuls of hp-1, keeping the scalar engine (exp) saturated
                pts_prev = None
                for hpi in range(KT + 1):
                    pts_cur = scores_block(hpi) if hpi < KT else None
                    if hpi >= 1:
                        av_block(hpi - 1, pts_prev)
                    pts_prev = pts_cur

            # ---------------- normalize + proj, fused per qc half
            with tc.tile_pool(name="wpp", bufs=KT) as wpp:
                wp = []
                for c in range(KT):
                    wt = wpp.tile([128, C], bf16, tag="wp", name=f"wp{c}")
                    nc.sync.dma_start(wt[:], wproj_d[c * 128 : (c + 1) * 128, :])
                    wp.append(wt)
                # token-major proj (aT chunks stationary) fused with the
                # residual + LN2 + transpose chain, staggered so the PE never
                # waits on a tile's vector chain
                h2Th = [
                    chain.tile(
                        [128, KT * 512], f8, tag=f"h2T{h}", bufs=1, name=f"h2T{h}"
                    )
                    for h in range(2)
                ]
                h2T3h = [t[:].rearrange("p (c t) -> p c t", t=512) for t in h2Th]
                with (
                    tc.tile_pool(name="psE", bufs=3, space="PSUM") as psE,
                    tc.tile_pool(name="psF", bufs=2, space="PSUM") as psF,
                ):
                    h2s = [None] * NT

                    def proj_tile(t):
                        pse = psE.tile([128, C], f32, tag="pse", name=f"pse{t}")
                        for hp in range(KT):
                            for fo, fw in ((0, 512), (512, 256)):
                                nc.tensor.matmul(
                                    pse[:, fo : fo + fw],
                                    aT[hp][:, t * 128 : (t + 1) * 128],
                                    wp[hp][:, fo : fo + fw],
                                    start=(hp == 0),
                                    stop=False,
                                )
                        for fo, fw in ((0, 512), (512, 256)):
                            nc.tensor.matmul(
                                pse[:, fo : fo + fw],
                                onesP[:],
                                bprow[:, fo : fo + fw],
                                start=False,
                                stop=True,
                            )
                        nc.vector.tensor_add(xs[t][:], xs[t][:], pse[:])
                        h2 = chain.tile([128, C], bf16, tag="hln", bufs=3, name=f"h2_{t}")
                        layernorm(h2[:], xs[t][:])
                        h2s[t] = h2

                    def h2_tr(t):
                        ps2 = psF.tile([128, C], bf16, tag="tp", name="psf2")
                        for ct in range(KT):
                            nc.tensor.transpose(
                                ps2[:, ct * 128 : (ct + 1) * 128],
                                h2s[t][:, ct * 128 : (ct + 1) * 128],
                                identb[:],
                            )
                        nc.scalar.activation(
                            h2T3h[t // 4][:, :, (t % 4) * 128 : (t % 4 + 1) * 128],
                            ps2[:].rearrange("p (c q) -> p c q", q=128),
                            AF.Copy,
                        )

                    for t in range(NT + 2):
                        if t < NT:
                            proj_tile(t)
                        if t >= 2:
                            h2_tr(t - 2)

            # ---------------- phase H: MLP + final residual + store
            with (
                tc.tile_pool(name="w1p", bufs=4) as w1p,
                tc.tile_pool(name="w2p", bufs=4) as w2p,
                tc.tile_pool(name="grp", bufs=4) as grp,
                tc.tile_pool(name="o2p", bufs=7) as o2p,
                tc.tile_pool(name="obp", bufs=3) as obp,
            ):
                DR = mybir.MatmulPerfMode.DoubleRow
                for qc in range(2):
                    with tc.tile_pool(name="psO", bufs=6, space="PSUM") as ps_o:
                        pso = [
                            ps_o.tile([128, 512], f32, tag="pso", name=f"pso{qc}_{i}")
                            for i in range(KT)
                        ]
                        with tc.tile_pool(name="psG2", bufs=2, space="PSUM") as ps_g:
                            gr2 = None
                            for r in range(RT):
                                w1 = w1p.tile([128, C], f8, tag="w1", name=f"w1_{r}")
                                nc.sync.dma_start(w1[:], wfc1_d[r])
                                w1v = w1[:].rearrange("p (k j) -> p k j", j=128)
                                if r % 2 == 0:
                                    w2 = w2p.tile(
                                        [128, 2 * C], f8, tag="w2", name=f"w2_{r}"
                                    )
                                    nc.sync.dma_start(
                                        w2[:].rearrange("p (i f) -> p i f", i=2),
                                        wfc2_d[r * 128 : (r + 2) * 128, :].rearrange(
                                            "(i p) f -> p i f", p=128
                                        ),
                                    )
                                    w2v = w2[:].rearrange("p (i f) -> p i f", i=2)
                                    gr2 = grp.tile(
                                        [128, 1024], f8, tag="gr", name=f"gr{r}"
                                    )
                                    gr2v = gr2[:].rearrange("p (i j) -> p i j", i=2)
                                psg = ps_g.tile([128, 512], f32, tag="psg", name="psg")
                                for ko in range(3):
                                    nc.tensor.matmul(
                                        psg[:],
                                        w1v[:, 2 * ko : 2 * ko + 2, :],
                                        h2T3h[qc][:, 2 * ko : 2 * ko + 2, :],
                                        start=(ko == 0),
                                        stop=(ko == 2),
                                        perf_mode=DR,
                                    )
                                nc.scalar.activation(
                                    gr2v[:, r % 2, :],
                                    psg[:],
                                    AF.Gelu,
                                    bias=bfc1_sb[:, r : r + 1],
                                    scale=1.0 / FP8S,
                                )
                                if r % 2 == 1:
                                    for co in range(KT):
                                        nc.tensor.matmul(
                                            pso[co][:],
                                            w2v[:, :, co * 128 : (co + 1) * 128],
                                            gr2v[:],
                                            start=(r == 1),
                                            stop=(r == RT - 1),
                                            perf_mode=DR,
                                        )
                        o2 = []
                        for co in range(KT):
                            o2t = o2p.tile([128, 512], bf16, tag="o2", name=f"o2_{qc}_{co}")
                            nc.vector.tensor_scalar(
                                o2t[:],
                                pso[co][:],
                                1.0 / FP8S,
                                bfc2_sb[:, co : co + 1],
                                op0=OP.mult,
                                op1=OP.add,
                            )
                            o2.append(o2t)
                    with tc.tile_pool(name="psH", bufs=2, space="PSUM") as psH:
                        for t4 in range(4):
                            t = qc * 4 + t4
                            ob = obp.tile([128, C], f32, tag="ob", name="ob")
                            ps = psH.tile([128, C], bf16, tag="tp", name="psh")
                            for co in range(KT):
                                nc.tensor.transpose(
                                    ps[:, co * 128 : (co + 1) * 128],
                                    o2[co][:, t4 * 128 : (t4 + 1) * 128],
                                    identb[:],
                                )
                            nc.vector.tensor_add(ob[:], xs[t][:], ps[:])
                            nc.sync.dma_start(out_d[t * 128 : (t + 1) * 128, :], ob[:])

    nc.compile()
    return nc


def _get_nc(reps=1):
    key = f"nc{reps}"
    if key not in _NC_CACHE:
        _NC_CACHE[key] = _build_nc(reps)
    return _NC_CACHE[key]


def _host_prep(inputs):
    import ml_dtypes

    inp = {k: np.asarray(v) for k, v in inputs.items()}
    x = np.ascontiguousarray(inp["x"], dtype=np.float32)  # [8, 1024, 768]
    g1 = inp["ln1_g"].astype(np.float64)
    b1 = inp["ln1_b"].astype(np.float64)
    qkv_w = inp["qkv_w"].astype(np.float64)  # [2304, 768]
    Ws = qkv_w.copy()
    Ws[:C] *= D ** (-0.5)  # fold attention scale into Wq
    wqkvT = np.ascontiguousarray((Ws * g1[None, :]).T).astype(
        ml_dtypes.bfloat16
    )  # [768, 2304]
    bqkv64 = Ws @ b1
    bqkv = bqkv64.astype(np.float32).reshape(1, 3 * C)

    # v-bias is a per-feature constant after softmax normalization; fold its
    # image under proj into the proj bias.
    proj_w64 = inp["proj_w"].astype(np.float64)
    wprojT = np.ascontiguousarray(proj_w64.T).astype(ml_dtypes.bfloat16)  # [768, 768]
    bproj = (inp["proj_b"].astype(np.float64) + proj_w64 @ bqkv64[2 * C :]).astype(
        np.float32
    ).reshape(1, C)

    g2 = inp["ln2_g"].astype(np.float64)
    b2 = inp["ln2_b"].astype(np.float64)
    fc1_w = inp["fc1_w"].astype(np.float64)  # [3072, 768]
    wfc1T = (fc1_w * g2[None, :]).T.astype(np.float32)  # [768, 3072]
    # pre-tiled: wfc1t[r, p, ct*128+j] = wfc1T[ct*128+p, r*128+j]; scaled by
    # FP8S to keep e4m3 values in the normal range (compensated on device)
    wfc1t = np.ascontiguousarray(
        wfc1T.reshape(KT, 128, RT, 128).transpose(2, 1, 0, 3).reshape(RT, 128, C)
        * FP8S
    ).astype(ml_dtypes.float8_e4m3)
    bfc1 = (fc1_w @ b2 + inp["fc1_b"].astype(np.float64)).astype(np.float32)
    bfc1 = bfc1.reshape(1, F1)
    wfc2T = np.ascontiguousarray(inp["fc2_w"].astype(np.float32).T * FP8S).astype(
        ml_dtypes.float8_e4m3
    )  # [3072, 768]
    bfc2 = inp["fc2_b"].astype(np.float32).reshape(1, C)

    # rel-bias toeplitz band: rband[h, p, w] = rel_table[clip(p + 1087 - w, 0, 128), h]
    tab = inp["rel_table"].astype(np.float32)  # [129, 12]
    p_i = np.arange(128)
    w_i = np.arange(W)
    idx = np.clip(p_i[:, None] + (N + 63) - w_i[None, :], 0, 2 * 64)
    rband = np.ascontiguousarray(tab[idx, :].transpose(2, 0, 1)).astype(
        ml_dtypes.bfloat16
    )  # [12, 128, 2047]

    sel = np.zeros((2, 128), np.float32)
    sel[0, 0:64] = 1.0
    sel[1, 64:128] = 1.0
    shared = {
        "sel": sel,
        "wqkvT": wqkvT,
        "bqkv": bqkv,
        "wprojT": wprojT,
        "bproj": bproj,
        "wfc1t": wfc1t,
        "bfc1": bfc1,
        "wfc2T": wfc2T,
        "bfc2": bfc2,
        "rband": rband,
    }
    in_maps = [{"x": np.ascontiguousarray(x[c]), **shared} for c in range(B)]
    return in_maps


def _make_runner(reps=1):
    import jax
    from jax.experimental.shard_map import shard_map
    from jax.sharding import Mesh, NamedSharding, PartitionSpec

    from concourse import bass2jax, mybir

    nc = _get_nc(reps)
    bass2jax.install_neuronx_cc_hook()

    partition_name = nc.partition_id_tensor.name if nc.partition_id_tensor else None
    in_names, out_names, out_avals, zero_outs = [], [], [], []
    for alloc in nc.m.functions[0].allocations:
        if not isinstance(alloc, mybir.MemoryLocationSet):
            continue
        name = alloc.memorylocations[0].name
        if alloc.kind == "ExternalInput":
            if name != partition_name:
                in_names.append(name)
        elif alloc.kind == "ExternalOutput":
            out_names.append(name)
            shape = tuple(alloc.tensor_shape)
            dtype = mybir.dt.np(alloc.dtype)
            out_avals.append(jax.core.ShapedArray(shape, dtype))
            zero_outs.append(np.zeros(shape, dtype))
    n_params = len(in_names)
    all_names = tuple(in_names) + tuple(out_names)
    if partition_name is not None:
        all_names = all_names + (partition_name,)
    donate = tuple(range(n_params, n_params + len(out_names)))

    def _body(*args):
        operands = list(args)
        if partition_name is not None:
            operands.append(bass2jax.partition_id_tensor())
        outs = bass2jax._bass_exec_p.bind(
            *operands,
            out_avals=tuple(out_avals),
            in_names=all_names,
            out_names=tuple(out_names),
            lowering_input_output_aliases=(),
            sim_require_finite=True,
            sim_require_nnan=True,
            nc=nc,
        )
        return tuple(outs)

    def _body_k(k):
        def body(*args):
            ins = list(args[:n_params])
            outs = list(args[n_params:])
            for _ in range(k):
                outs = list(_body(*ins, *outs))
            return tuple(outs)

        return body

    devices = jax.devices()[:B]
    mesh = Mesh(np.asarray(devices), ("core",))
    in_specs = (PartitionSpec("core"),) * (n_params + len(out_names))
    out_specs = (PartitionSpec("core"),) * len(out_names)

    def make_fn(k):
        return jax.jit(
            shard_map(
                _body_k(k),
                mesh=mesh,
                in_specs=in_specs,
                out_specs=out_specs,
                check_rep=False,
            ),
            donate_argnums=donate,
            keep_unused=True,
        )

    sharding = NamedSharding(mesh, PartitionSpec("core"))
    return make_fn, in_names, out_names, zero_outs, sharding


def _get_runner(reps=1):
    key = f"runner{reps}"
    if key not in _NC_CACHE:
        _NC_CACHE[key] = _make_runner(reps)
    return _NC_CACHE[key]


LAST_BENCH = None


def kernel(**inputs):
    global LAST_BENCH
    import time

    import jax

    make_fn, in_names, out_names, zero_outs, sharding = _get_runner()
    in_maps = _host_prep(inputs)
    concat_in = [
        np.concatenate([np.asarray(in_maps[c][n]) for c in range(B)], axis=0)
        for n in in_names
    ]
    concat_zeros = [
        np.zeros((B * z.shape[0], *z.shape[1:]), z.dtype) for z in zero_outs
    ]
    fn1 = make_fn(1)
    dev_in = [jax.device_put(a, sharding) for a in concat_in]
    outs = fn1(*dev_in, *concat_zeros)
    jax.block_until_ready(outs)
    result = np.asarray(outs[0]).reshape(B, N, C).astype(np.float32)

    iters = int(os.environ.get("BENCH_ITERS", "0"))
    if iters > 0:
        o = fn1(*dev_in, *outs)  # warm
        jax.block_until_ready(o)
        times = []
        for _ in range(iters):
            t0 = time.perf_counter()
            o = fn1(*dev_in, *o)
            jax.block_until_ready(o)
            times.append(time.perf_counter() - t0)
        overhead = _bench_overhead()
        t_min = float(np.min(times))
        t_med = float(np.median(times))
        LAST_BENCH = {
            "per_iter_ns": max(t_min - overhead, 0.0) * 1e9,
            "call_min_ns": t_min * 1e9,
            "call_med_ns": t_med * 1e9,
            "overhead_ns": overhead * 1e9,
            "iters": iters,
        }
    return result


def _bench_overhead():
    """Per-call dispatch overhead, measured with a trivial 1-DMA kernel."""
    import time

    import jax
    from jax.experimental.shard_map import shard_map
    from jax.sharding import Mesh, PartitionSpec

    import concourse.bacc as bacc
    import concourse.tile as tile
    from concourse import bass2jax, mybir

    if "tiny" not in _NC_CACHE:
        f32 = mybir.dt.float32
        nc = bacc.Bacc(
            "TRN2",
            target_bir_lowering=False,
            debug=False,
            enable_asserts=False,
            num_devices=8,
        )
        xi = nc.dram_tensor("ti", [128, 128], f32, kind="ExternalInput").ap()
        xo = nc.dram_tensor("to", [128, 128], f32, kind="ExternalOutput").ap()
        with tile.TileContext(nc) as tc:
            with tc.tile_pool(name="p", bufs=1) as p:
                t = p.tile([128, 128], f32, tag="t", name="t")
                nc.sync.dma_start(t[:], xi[:])
                nc.sync.dma_start(xo[:], t[:])
        nc.compile()

        partition_name = nc.partition_id_tensor.name if nc.partition_id_tensor else None
        all_names = ["ti", "to"]
        if partition_name is not None:
            all_names.append(partition_name)
        out_avals = [jax.core.ShapedArray((128, 128), np.float32)]

        def _tbody(*args):
            operands = list(args)
            if partition_name is not None:
                operands.append(bass2jax.partition_id_tensor())
            return tuple(
                bass2jax._bass_exec_p.bind(
                    *operands,
                    out_avals=tuple(out_avals),
                    in_names=tuple(all_names),
                    out_names=("to",),
                    lowering_input_output_aliases=(),
                    sim_require_finite=True,
                    sim_require_nnan=True,
                    nc=nc,
                )
            )

        devices = jax.devices()[:B]
        mesh = Mesh(np.asarray(devices), ("core",))
        tfn = jax.jit(
            shard_map(
                _tbody,
                mesh=mesh,
                in_specs=(PartitionSpec("core"),) * 2,
                out_specs=(PartitionSpec("core"),),
                check_rep=False,
            ),
            donate_argnums=(1,),
            keep_unused=True,
        )
        _NC_CACHE["tiny"] = tfn

    tfn = _NC_CACHE["tiny"]
    ti = np.zeros((B * 128, 128), np.float32)
    o = tfn(ti, np.zeros((B * 128, 128), np.float32))
    jax.block_until_ready(o)
    times = []
    for _ in range(30):
        t0 = time.perf_counter()
        o = tfn(ti, *([o] if not isinstance(o, tuple) else list(o)))
        jax.block_until_ready(o)
        times.append(time.perf_counter() - t0)
    return float(np.min(times))

